# revision 1
# baseline (speedup 1.0000x reference)
"""Distributed HGNN+ convolution for 8 Trainium2 NeuronCores (Bass/Tile).

Math (dense hypergraph incidence H [N_V, N_E], features X [N_V, C]):
    Xt  = X @ W.T + b                    # theta
    Xe  = (H.T @ Xt) * 1/colsum(H)       # V2E mean aggregation
    Xv  = (H @ Xe)   * 1/rowsum(H)       # E2V mean aggregation
    out = relu(Xv)

Distribution: vertex rows are sharded across the 8 cores (per the
row-parallel scheme): each core computes theta on its vertex shard, a
partial V2E GEMM, one bf16 AllReduce of the partial edge features, then a
fully row-parallel E2V GEMM over its own vertex rows.

The degree vectors ride along for free: a constant 'ones' channel is
inserted at column C/2 of the matmul RHS. Since the 512-wide moving
operand must be split into two <=512-column matmuls anyway, making the
split (257 | 256) adds a single extra column of streaming and produces
colsum(H) inside the all-reduced V2E output (and rowsum(H) inside the E2V
output). Scaling the all-reduced buffer by 1/colsum turns that column
into exactly the 'ones' column the next pass needs.

Compute is bf16 with fp32 PSUM accumulation (well within the 2e-2
relative-error envelope); the all-reduce is bf16, chunked 4x so it
overlaps the tail of the V2E GEMM.
"""

import contextlib

import numpy as np
import ml_dtypes

BF16 = ml_dtypes.bfloat16

# Problem shape (hardcoded per contract).
N_V, N_E, CH, NCORES = 16384, 8192, 512, 8


def _full_cfg():
    # nchunks=1: one big AllReduce. HW slope-timing showed each chunked
    # collective pays a ~200us fixed floor that dwarfs its bandwidth cost,
    # so fewer chunks beat the extra V2E/AR overlap of nchunks=4.
    return dict(n_v=N_V, n_e=N_E, ch=CH, ncores=NCORES, nchunks=1)


def build_graph(tc, io, cfg):
    """Emit the Tile IR. io: dict of DRAM APs: hsp, htp, xta, wtb, out."""
    from concourse import mybir

    nc = tc.nc
    f32 = mybir.dt.float32
    bf16 = mybir.dt.bfloat16
    Relu = mybir.ActivationFunctionType.Relu

    n_v, n_e, ch, ncores, nch = (
        cfg["n_v"], cfg["n_e"], cfg["ch"], cfg["ncores"], cfg["nchunks"],
    )
    VS = n_v // ncores      # vertices per core
    KV = VS // 128          # vertex 128-tiles per core
    EM = n_e // 128         # edge 128-tiles (global)
    CK = ch // 128          # theta contraction tiles over in-channels
    S2 = ch // 2            # second half width
    S1 = S2 + 1             # first half + the ones/degree column
    SW = ch + 1             # augmented row width
    # SBUF stride for xe tiles, padded to a 32-byte multiple: DMA writes of
    # adjacent unaligned tiles RMW shared 32B beats and race (HW-observed:
    # the last bf16 element of each 1026B tile got stale bytes).
    SWP = (SW + 15) // 16 * 16
    EMC = EM // nch         # edge tiles per all-reduce chunk
    rg = [list(range(ncores))]

    hsp, htp, xta, wtb, out = io["hsp"], io["htp"], io["xta"], io["wtb"], io["out"]

    with contextlib.ExitStack() as ctx:
        theta_in = ctx.enter_context(tc.tile_pool(name="theta_in", bufs=1))
        xt_pool = ctx.enter_context(tc.tile_pool(name="xt_pool", bufs=1))
        xe_pool = ctx.enter_context(tc.tile_pool(name="xe_pool", bufs=1))
        hs_pool = ctx.enter_context(tc.tile_pool(name="hs_pool", bufs=3))
        ht_pool = ctx.enter_context(tc.tile_pool(name="ht_pool", bufs=2))
        sb_out = ctx.enter_context(tc.tile_pool(name="sb_out", bufs=3))
        rec_pool = ctx.enter_context(tc.tile_pool(name="rec_pool", bufs=4))
        psum = ctx.enter_context(tc.tile_pool(name="psum", bufs=2, space="PSUM"))
        dram = ctx.enter_context(tc.tile_pool(name="dram", bufs=1, space="DRAM"))

        # ---- theta: Xt_aug = [X | 1 | 0pad] @ [W.T ; b ; 0pad], kept in SBUF
        # as KV tiles of [128 v, SW] with the ones column at S2. The inputs
        # are zero-padded to CKT full 128-row contraction tiles (the ones/bias
        # rank-1 bias update rides in tile CK).
        CKT = CK + 1
        xta_sb = theta_in.tile([128, CKT * VS], bf16)
        nc.sync.dma_start(
            xta_sb.rearrange("p (k f) -> p k f", k=CKT),
            xta.rearrange("(k p) f -> p k f", p=128),
        )
        wtb_sb = theta_in.tile([128, CKT * ch], bf16)
        nc.sync.dma_start(
            wtb_sb.rearrange("p (k f) -> p k f", k=CKT),
            wtb.rearrange("(k p) f -> p k f", p=128),
        )

        xt_all = xt_pool.tile([128, KV * SW], bf16)

        for vm in range(KV):
            ps = psum.tile([128, ch], f32, tag="ps_theta", name="ps_theta")
            for kt in range(CKT):
                nc.tensor.matmul(
                    ps,
                    lhsT=xta_sb[:, kt * VS + vm * 128 : kt * VS + (vm + 1) * 128],
                    rhs=wtb_sb[:, kt * ch : (kt + 1) * ch],
                    start=(kt == 0),
                    stop=(kt == CKT - 1),
                )
            base = vm * SW
            nc.vector.tensor_copy(xt_all[:, base : base + S2], ps[:, 0:S2])
            nc.vector.memset(xt_all[:, base + S2 : base + S1], 1.0)
            nc.vector.tensor_copy(xt_all[:, base + S1 : base + SW], ps[:, S2:ch])

        # ---- V2E partial GEMM + chunked AllReduce + colsum scaling.
        arin = [
            dram.tile([EMC * 128, SW], bf16, name=f"arin{c}", tag=f"arin{c}")
            for c in range(nch)
        ]
        arout = [
            dram.tile([EMC * 128, SW], bf16, name=f"arout{c}", tag=f"arout{c}",
                      addr_space="Shared")
            for c in range(nch)
        ]
        xe_all = xe_pool.tile([128, EM * SWP], bf16)

        for em in range(EM):
            hs_sb = hs_pool.tile([128, KV * 128], bf16, tag="hs", name="hs_sb")
            nc.sync.dma_start(hs_sb, hsp[em])
            psA = psum.tile([128, S1], f32, tag="psA", name="psA")
            psB = psum.tile([128, S2], f32, tag="psB", name="psB")
            for kt in range(KV):
                w = hs_sb[:, kt * 128 : (kt + 1) * 128]
                xb = kt * SW
                nc.tensor.matmul(psA, lhsT=w, rhs=xt_all[:, xb : xb + S1],
                                 start=(kt == 0), stop=(kt == KV - 1))
                nc.tensor.matmul(psB, lhsT=w, rhs=xt_all[:, xb + S1 : xb + SW],
                                 start=(kt == 0), stop=(kt == KV - 1))
            ar_sb = sb_out.tile([128, SW], bf16, tag="ar_sb", name="ar_sb")
            nc.vector.tensor_copy(ar_sb[:, 0:S1], psA[:, :])
            nc.vector.tensor_copy(ar_sb[:, S1:SW], psB[:, :])
            c, j = divmod(em, EMC)
            nc.sync.dma_start(arin[c][j * 128 : (j + 1) * 128, :], ar_sb)

            if j == EMC - 1:
                if cfg.get("local_copy"):
                    # debug/profiling variant: no collective, plain DRAM copy
                    nc.sync.dma_start(arout[c][:, :], arin[c][:, :])
                else:
                    nc.gpsimd.collective_compute(
                        "AllReduce",
                        mybir.AluOpType.add,
                        replica_groups=rg,
                        ins=[arin[c].opt()],
                        outs=[arout[c].opt()],
                    )
                # Scale the reduced chunk by 1/colsum; the colsum column
                # itself becomes exactly 1.0 — the ones column for E2V.
                for jj in range(EMC):
                    ke = c * EMC + jj
                    xb = ke * SWP
                    nc.sync.dma_start(
                        xe_all[:, xb : xb + SW],
                        arout[c][jj * 128 : (jj + 1) * 128, :],
                    )
                    r = rec_pool.tile([128, 1], f32, tag="r", name="r")
                    nc.vector.reciprocal(r, xe_all[:, xb + S2 : xb + S1])
                    nc.vector.tensor_scalar_mul(
                        xe_all[:, xb : xb + SW], xe_all[:, xb : xb + SW], r
                    )

        # ---- E2V GEMM (row-parallel) + rowsum scaling + ReLU.
        for vm in range(KV):
            ht_sb = ht_pool.tile([128, EM * 128], bf16, tag="ht", name="ht_sb")
            nc.sync.dma_start(ht_sb, htp[vm])
            psA = psum.tile([128, S1], f32, tag="psA", name="psA2")
            psB = psum.tile([128, S2], f32, tag="psB", name="psB2")
            for ke in range(EM):
                w = ht_sb[:, ke * 128 : (ke + 1) * 128]
                xb = ke * SWP
                nc.tensor.matmul(psA, lhsT=w, rhs=xe_all[:, xb : xb + S1],
                                 start=(ke == 0), stop=(ke == EM - 1))
                nc.tensor.matmul(psB, lhsT=w, rhs=xe_all[:, xb + S1 : xb + SW],
                                 start=(ke == 0), stop=(ke == EM - 1))
            r = rec_pool.tile([128, 1], f32, tag="r", name="r2")
            nc.vector.reciprocal(r, psA[:, S2:S1])
            o_sb = sb_out.tile([128, ch], f32, tag="o_sb", name="o_sb")
            nc.scalar.activation(o_sb[:, 0:S2], psA[:, 0:S2], Relu, scale=r)
            nc.scalar.activation(o_sb[:, S2:ch], psB[:, :], Relu, scale=r)
            nc.sync.dma_start(out[vm * 128 : (vm + 1) * 128, :], o_sb)


def pack_inputs(X, H, W, b, cfg):
    """Host-side shard/cast/pack. Returns one input map per core."""
    from concurrent.futures import ThreadPoolExecutor

    n_v, n_e, ch, ncores = cfg["n_v"], cfg["n_e"], cfg["ch"], cfg["ncores"]
    VS = n_v // ncores
    KV = VS // 128
    EM = n_e // 128

    wtb = np.vstack(
        [
            np.ascontiguousarray(W.T).astype(np.float32),
            b[None, :].astype(np.float32),
            np.zeros((127, ch), np.float32),
        ]
    ).astype(BF16)

    H_bf = H.astype(BF16)

    def pack_core(c):
        Hc = H_bf[c * VS : (c + 1) * VS]
        R = Hc.reshape(KV, 128, EM, 128)
        # hsp[em, p, kt*128+f] = Hc[kt*128+p, em*128+f]  (V2E lhsT panels)
        hsp = np.ascontiguousarray(R.transpose(2, 1, 0, 3)).reshape(EM, 128, VS)
        # htp[vm, p, ke*128+f] = Hc[vm*128+f, ke*128+p]  (E2V lhsT panels)
        htp = np.ascontiguousarray(R.transpose(0, 3, 2, 1)).reshape(KV, 128, n_e)
        Xc = X[c * VS : (c + 1) * VS]
        xta = np.vstack(
            [
                np.ascontiguousarray(Xc.T),
                np.ones((1, VS), np.float32),
                np.zeros((127, VS), np.float32),
            ]
        ).astype(BF16)
        return dict(hsp=hsp, htp=htp, xta=xta, wtb=wtb)

    with ThreadPoolExecutor(max_workers=ncores) as ex:
        return list(ex.map(pack_core, range(ncores)))


_cache = {}


def _build_compiled(cfg, reps=1):
    key = (tuple(sorted(cfg.items())), reps)
    if key in _cache:
        return _cache[key]
    from concourse import bacc, mybir, tile

    n_v, n_e, ch, ncores = cfg["n_v"], cfg["n_e"], cfg["ch"], cfg["ncores"]
    VS = n_v // ncores
    KV = VS // 128
    EM = n_e // 128

    nc = bacc.Bacc("TRN2", target_bir_lowering=False, debug=False,
                   num_devices=ncores)
    io = {
        "hsp": nc.dram_tensor("hsp", [EM, 128, VS], mybir.dt.bfloat16,
                              kind="ExternalInput").ap(),
        "htp": nc.dram_tensor("htp", [KV, 128, n_e], mybir.dt.bfloat16,
                              kind="ExternalInput").ap(),
        "xta": nc.dram_tensor("xta", [ch + 128, VS], mybir.dt.bfloat16,
                              kind="ExternalInput").ap(),
        "wtb": nc.dram_tensor("wtb", [ch + 128, ch], mybir.dt.bfloat16,
                              kind="ExternalInput").ap(),
        "out": nc.dram_tensor("out", [VS, ch], mybir.dt.float32,
                              kind="ExternalOutput").ap(),
    }
    with tile.TileContext(nc) as tc:
        for _ in range(reps):
            build_graph(tc, io, cfg)
    nc.compile()
    _cache[key] = nc
    return nc


def kernel(X, H, W, b, _trace=False, _cfg=None):
    from concourse.bass_utils import run_bass_kernel_spmd

    cfg = _cfg or _full_cfg()
    X = np.asarray(X, dtype=np.float32)
    H = np.asarray(H, dtype=np.float32)
    W = np.asarray(W, dtype=np.float32)
    b = np.asarray(b, dtype=np.float32)

    nc = _build_compiled(cfg)
    in_maps = pack_inputs(X, H, W, b, cfg)
    res = run_bass_kernel_spmd(
        nc, in_maps, core_ids=list(range(cfg["ncores"])), trace=_trace
    )
    kernel.last_result = res
    return np.concatenate([r["out"] for r in res.results], axis=0)


kernel.last_result = None



# revision 3
# speedup vs baseline: 1.2794x; 1.2794x over previous
"""Distributed HGNN+ convolution for 8 Trainium2 NeuronCores (Bass/Tile).

Math (dense hypergraph incidence H [N_V, N_E], features X [N_V, C]):
    Xt  = X @ W.T + b                    # theta
    Xe  = (H.T @ Xt) * 1/colsum(H)       # V2E mean aggregation
    Xv  = (H @ Xe)   * 1/rowsum(H)       # E2V mean aggregation
    out = relu(Xv)

Distribution: vertex rows are sharded across the 8 cores. Each core
computes theta on its vertex shard, a partial V2E GEMM, an AllReduce of
the partial edge features, then a fully row-parallel E2V GEMM over its
own vertex rows.

Both degree scalings are folded into the E2V operand on the host:
    htp2 = diag(1/rowsum) @ H @ diag(1/colsum)
so the device graph is three plain GEMMs + ReLU: every matmul streams a
full 512-wide PSUM bank and no on-device reciprocals/rescales exist.

The AllReduce is split in two 4.2MB chunks to hide it entirely:
chunk 0 (edge tiles 0-31) fires at the V2E midpoint and completes under
the remaining V2E compute; chunk 1 fires at V2E end and completes under
E2V's chunk-0 compute. E2V accumulates per-edge-chunk partials into
SBUF fp32 accumulators so it can consume AR chunks in arrival order.

Compute is bf16 with fp32 PSUM accumulation; rel-err lands ~2e-3, well
within the 2e-2 envelope.
"""

import contextlib

import numpy as np
import ml_dtypes

BF16 = ml_dtypes.bfloat16

# Problem shape (hardcoded per contract).
N_V, N_E, CH, NCORES = 16384, 8192, 512, 8


def _full_cfg():
    return dict(n_v=N_V, n_e=N_E, ch=CH, ncores=NCORES, nchunks=2)


def build_graph(tc, io, cfg):
    """Emit the Tile IR. io: dict of DRAM APs: hsp, htp, xta, wtb, out."""
    from concourse import mybir

    nc = tc.nc
    f32 = mybir.dt.float32
    bf16 = mybir.dt.bfloat16
    Relu = mybir.ActivationFunctionType.Relu
    Add = mybir.AluOpType.add

    n_v, n_e, ch, ncores, nch = (
        cfg["n_v"], cfg["n_e"], cfg["ch"], cfg["ncores"], cfg["nchunks"],
    )
    VS = n_v // ncores      # vertices per core
    KV = VS // 128          # vertex 128-tiles per core
    EM = n_e // 128         # edge 128-tiles (global)
    CK = ch // 128          # theta contraction tiles over in-channels
    CKT = CK + 1            # + the ones/bias rank-1 tile
    EMC = EM // nch         # edge tiles per all-reduce chunk
    rg = [list(range(ncores))]

    hsp, htp, xta, wtb, out = io["hsp"], io["htp"], io["xta"], io["wtb"], io["out"]

    with contextlib.ExitStack() as ctx:
        theta_in = ctx.enter_context(tc.tile_pool(name="theta_in", bufs=1))
        xt_pool = ctx.enter_context(tc.tile_pool(name="xt_pool", bufs=1))
        xe_pool = ctx.enter_context(tc.tile_pool(name="xe_pool", bufs=1))
        acc_pool = ctx.enter_context(tc.tile_pool(name="acc_pool", bufs=1))
        hs_pool = ctx.enter_context(tc.tile_pool(name="hs_pool", bufs=3))
        ht_pool = ctx.enter_context(tc.tile_pool(name="ht_pool", bufs=3))
        sb_out = ctx.enter_context(tc.tile_pool(name="sb_out", bufs=3))
        psum = ctx.enter_context(tc.tile_pool(name="psum", bufs=3, space="PSUM"))
        dram = ctx.enter_context(tc.tile_pool(name="dram", bufs=1, space="DRAM"))

        # ---- theta: Xt = [X ; 1 ; 0pad].T @ [W.T ; b ; 0pad], kept in SBUF
        # as KV tiles of [128 v, ch]. Inputs are zero-padded to CKT full
        # 128-row contraction tiles (the ones/bias rank-1 update is tile CK).
        xta_sb = theta_in.tile([128, CKT * VS], bf16)
        nc.sync.dma_start(
            xta_sb.rearrange("p (k f) -> p k f", k=CKT),
            xta.rearrange("(k p) f -> p k f", p=128),
        )
        wtb_sb = theta_in.tile([128, CKT * ch], bf16)
        nc.sync.dma_start(
            wtb_sb.rearrange("p (k f) -> p k f", k=CKT),
            wtb.rearrange("(k p) f -> p k f", p=128),
        )

        xt_all = xt_pool.tile([128, KV * ch], bf16)

        for vm in range(KV):
            ps = psum.tile([128, ch], f32, tag="ps", name="ps_theta")
            for kt in range(CKT):
                nc.tensor.matmul(
                    ps,
                    lhsT=xta_sb[:, kt * VS + vm * 128 : kt * VS + (vm + 1) * 128],
                    rhs=wtb_sb[:, kt * ch : (kt + 1) * ch],
                    start=(kt == 0),
                    stop=(kt == CKT - 1),
                )
            nc.vector.tensor_copy(xt_all[:, vm * ch : (vm + 1) * ch], ps)

        # ---- V2E partial GEMM + chunked AllReduce.
        arin = [
            dram.tile([EMC * 128, ch], bf16, name=f"arin{c}", tag=f"arin{c}")
            for c in range(nch)
        ]
        arout = [
            dram.tile([EMC * 128, ch], bf16, name=f"arout{c}", tag=f"arout{c}",
                      addr_space="Shared")
            for c in range(nch)
        ]
        xe_all = xe_pool.tile([128, EM * ch], bf16)

        for em in range(EM):
            hs_sb = hs_pool.tile([128, VS], bf16, tag="hs", name="hs_sb")
            nc.sync.dma_start(hs_sb, hsp[em])
            ps = psum.tile([128, ch], f32, tag="ps", name="ps_v2e")
            for kt in range(KV):
                nc.tensor.matmul(
                    ps,
                    lhsT=hs_sb[:, kt * 128 : (kt + 1) * 128],
                    rhs=xt_all[:, kt * ch : (kt + 1) * ch],
                    start=(kt == 0),
                    stop=(kt == KV - 1),
                )
            ar_sb = sb_out.tile([128, ch], bf16, tag="ar_sb", name="ar_sb")
            nc.vector.tensor_copy(ar_sb, ps)
            c, j = divmod(em, EMC)
            nc.sync.dma_start(arin[c][j * 128 : (j + 1) * 128, :], ar_sb)

            if j == EMC - 1:
                nc.gpsimd.collective_compute(
                    "AllReduce",
                    mybir.AluOpType.add,
                    replica_groups=rg,
                    ins=[arin[c].opt()],
                    outs=[arout[c].opt()],
                )
                for jj in range(EMC):
                    ke = c * EMC + jj
                    nc.sync.dma_start(
                        xe_all[:, ke * ch : (ke + 1) * ch],
                        arout[c][jj * 128 : (jj + 1) * 128, :],
                    )

        # ---- E2V GEMM (row-parallel), chunk-major over the AR chunks so
        # chunk c's compute overlaps AR chunk c+1. Per-vm fp32 accumulators
        # live in SBUF; the degree scalings are already folded into htp.
        acc = acc_pool.tile([128, KV * ch], f32)
        for c in range(nch):
            for vm in range(KV):
                ht_sb = ht_pool.tile([128, EMC * 128], bf16, tag="ht", name="ht_sb")
                nc.sync.dma_start(ht_sb, htp[vm][:, c * EMC * 128 : (c + 1) * EMC * 128])
                ps = psum.tile([128, ch], f32, tag="ps2", name="ps_e2v")
                for kk in range(EMC):
                    ke = c * EMC + kk
                    nc.tensor.matmul(
                        ps,
                        lhsT=ht_sb[:, kk * 128 : (kk + 1) * 128],
                        rhs=xe_all[:, ke * ch : (ke + 1) * ch],
                        start=(kk == 0),
                        stop=(kk == EMC - 1),
                    )
                a = acc[:, vm * ch : (vm + 1) * ch]
                if c == 0:
                    nc.vector.tensor_copy(a, ps)
                elif c < nch - 1:
                    nc.vector.tensor_tensor(a, a, ps, op=Add)
                else:
                    s_sb = sb_out.tile([128, ch], f32, tag="s_sb", name="s_sb")
                    nc.vector.tensor_tensor(s_sb, a, ps, op=Add)
                    o_sb = sb_out.tile([128, ch], f32, tag="o_sb", name="o_sb")
                    nc.scalar.activation(o_sb, s_sb, Relu)
                    nc.sync.dma_start(out[vm * 128 : (vm + 1) * 128, :], o_sb)


def pack_inputs(X, H, W, b, cfg):
    """Host-side shard/cast/pack. Returns one input map per core."""
    from concurrent.futures import ThreadPoolExecutor

    n_v, n_e, ch, ncores = cfg["n_v"], cfg["n_e"], cfg["ch"], cfg["ncores"]
    VS = n_v // ncores
    KV = VS // 128
    EM = n_e // 128

    wtb = np.vstack(
        [
            np.ascontiguousarray(W.T).astype(np.float32),
            b[None, :].astype(np.float32),
            np.zeros((127, ch), np.float32),
        ]
    ).astype(BF16)

    # Degree scalings, folded into the E2V operand (matches the reference's
    # safe-reciprocal semantics: zero degree -> zero row/col scale).
    colsum = H.sum(axis=0, dtype=np.float32)
    rowsum = H.sum(axis=1, dtype=np.float32)
    de_inv = np.where(colsum == 0, 0.0, 1.0 / colsum).astype(np.float32)
    dv_inv = np.where(rowsum == 0, 0.0, 1.0 / rowsum).astype(np.float32)

    H_bf = H.astype(BF16)

    def pack_core(c):
        Hc = H_bf[c * VS : (c + 1) * VS]
        R = Hc.reshape(KV, 128, EM, 128)
        # hsp[em, p, kt*128+f] = Hc[kt*128+p, em*128+f]  (V2E lhsT panels)
        hsp = np.ascontiguousarray(R.transpose(2, 1, 0, 3)).reshape(EM, 128, VS)
        # htp[vm, p, ke*128+f] = Hs[vm*128+f, ke*128+p]  (E2V lhsT panels,
        # with both degree scalings pre-applied)
        Hs = (
            dv_inv[c * VS : (c + 1) * VS, None]
            * H[c * VS : (c + 1) * VS]
            * de_inv[None, :]
        ).astype(BF16)
        R2 = Hs.reshape(KV, 128, EM, 128)
        htp = np.ascontiguousarray(R2.transpose(0, 3, 2, 1)).reshape(KV, 128, n_e)
        Xc = X[c * VS : (c + 1) * VS]
        xta = np.vstack(
            [
                np.ascontiguousarray(Xc.T),
                np.ones((1, VS), np.float32),
                np.zeros((127, VS), np.float32),
            ]
        ).astype(BF16)
        return dict(hsp=hsp, htp=htp, xta=xta, wtb=wtb)

    with ThreadPoolExecutor(max_workers=ncores) as ex:
        return list(ex.map(pack_core, range(ncores)))


_cache = {}


def _build_compiled(cfg, reps=1):
    key = (tuple(sorted(cfg.items())), reps)
    if key in _cache:
        return _cache[key]
    from concourse import bacc, mybir, tile

    n_v, n_e, ch, ncores = cfg["n_v"], cfg["n_e"], cfg["ch"], cfg["ncores"]
    VS = n_v // ncores
    KV = VS // 128
    EM = n_e // 128

    nc = bacc.Bacc("TRN2", target_bir_lowering=False, debug=False,
                   num_devices=ncores)
    io = {
        "hsp": nc.dram_tensor("hsp", [EM, 128, VS], mybir.dt.bfloat16,
                              kind="ExternalInput").ap(),
        "htp": nc.dram_tensor("htp", [KV, 128, n_e], mybir.dt.bfloat16,
                              kind="ExternalInput").ap(),
        "xta": nc.dram_tensor("xta", [ch + 128, VS], mybir.dt.bfloat16,
                              kind="ExternalInput").ap(),
        "wtb": nc.dram_tensor("wtb", [ch + 128, ch], mybir.dt.bfloat16,
                              kind="ExternalInput").ap(),
        "out": nc.dram_tensor("out", [VS, ch], mybir.dt.float32,
                              kind="ExternalOutput").ap(),
    }
    with tile.TileContext(nc) as tc:
        for _ in range(reps):
            build_graph(tc, io, cfg)
    nc.compile()
    _cache[key] = nc
    return nc


def kernel(X, H, W, b, _trace=False, _cfg=None):
    from concourse.bass_utils import run_bass_kernel_spmd

    cfg = _cfg or _full_cfg()
    X = np.asarray(X, dtype=np.float32)
    H = np.asarray(H, dtype=np.float32)
    W = np.asarray(W, dtype=np.float32)
    b = np.asarray(b, dtype=np.float32)

    nc = _build_compiled(cfg)
    in_maps = pack_inputs(X, H, W, b, cfg)
    res = run_bass_kernel_spmd(
        nc, in_maps, core_ids=list(range(cfg["ncores"])), trace=_trace
    )
    kernel.last_result = res
    return np.concatenate([r["out"] for r in res.results], axis=0)


kernel.last_result = None


# revision 7
# speedup vs baseline: 1.3075x; 1.0220x over previous
"""Distributed HGNN+ convolution for 8 Trainium2 NeuronCores (Bass/Tile).

Math (dense hypergraph incidence H [N_V, N_E], features X [N_V, C]):
    Xt  = X @ W.T + b                    # theta
    Xe  = (H.T @ Xt) * 1/colsum(H)       # V2E mean aggregation
    Xv  = (H @ Xe)   * 1/rowsum(H)       # E2V mean aggregation
    out = relu(Xv)

Distribution: vertex rows are sharded across the 8 cores. Each core
computes theta on its vertex shard, a partial V2E GEMM, an AllReduce of
the partial edge features, then a fully row-parallel E2V GEMM over its
own vertex rows.

Both degree scalings are folded into the E2V operand on the host:
    htp2 = diag(1/rowsum) @ H @ diag(1/colsum)
so the device graph is three plain GEMMs + ReLU: every matmul streams a
full 512-wide PSUM bank and no on-device reciprocals/rescales exist.

The AllReduce is split in two 4.2MB chunks to hide it entirely:
chunk 0 (edge tiles 0-31) fires at the V2E midpoint and completes under
the remaining V2E compute; chunk 1 fires at V2E end and completes under
E2V's chunk-0 compute. E2V accumulates per-edge-chunk partials into
SBUF fp32 accumulators so it can consume AR chunks in arrival order.

Compute is bf16 with fp32 PSUM accumulation; rel-err lands ~2e-3, well
within the 2e-2 envelope.
"""

import contextlib

import numpy as np
import ml_dtypes

BF16 = ml_dtypes.bfloat16

# Problem shape (hardcoded per contract).
N_V, N_E, CH, NCORES = 16384, 8192, 512, 8


def _full_cfg():
    return dict(n_v=N_V, n_e=N_E, ch=CH, ncores=NCORES, nchunks=2)


def build_graph(tc, io, cfg):
    """Emit the Tile IR. io: dict of DRAM APs: hsp, htp, xta, wtb, out."""
    from concourse import mybir

    nc = tc.nc
    f32 = mybir.dt.float32
    bf16 = mybir.dt.bfloat16
    Relu = mybir.ActivationFunctionType.Relu
    Add = mybir.AluOpType.add

    n_v, n_e, ch, ncores, nch = (
        cfg["n_v"], cfg["n_e"], cfg["ch"], cfg["ncores"], cfg["nchunks"],
    )
    VS = n_v // ncores      # vertices per core
    KV = VS // 128          # vertex 128-tiles per core
    EM = n_e // 128         # edge 128-tiles (global)
    CK = ch // 128          # theta contraction tiles over in-channels
    CKT = CK + 1            # + the ones/bias rank-1 tile
    EMC = EM // nch         # edge tiles per all-reduce chunk
    rg = [list(range(ncores))]

    hsp, htp, xta, wtb, out = io["hsp"], io["htp"], io["xta"], io["wtb"], io["out"]

    with contextlib.ExitStack() as ctx:
        theta_in = ctx.enter_context(tc.tile_pool(name="theta_in", bufs=1))
        xt_pool = ctx.enter_context(tc.tile_pool(name="xt_pool", bufs=1))
        xe_pool = ctx.enter_context(tc.tile_pool(name="xe_pool", bufs=1))
        acc_pool = ctx.enter_context(tc.tile_pool(name="acc_pool", bufs=1))
        hs_pool = ctx.enter_context(tc.tile_pool(name="hs_pool", bufs=4))
        ht_pool = ctx.enter_context(tc.tile_pool(name="ht_pool", bufs=3))
        sb_out = ctx.enter_context(tc.tile_pool(name="sb_out", bufs=2))
        psum = ctx.enter_context(tc.tile_pool(name="psum", bufs=3, space="PSUM"))
        dram = ctx.enter_context(tc.tile_pool(name="dram", bufs=1, space="DRAM"))

        # ---- theta: Xt = [X ; 1 ; 0pad].T @ [W.T ; b ; 0pad], kept in SBUF
        # as KV tiles of [128 v, ch]. Inputs are zero-padded to CKT full
        # 128-row contraction tiles (the ones/bias rank-1 update is tile CK).
        xta_sb = theta_in.tile([128, CKT * VS], bf16)
        nc.sync.dma_start(
            xta_sb.rearrange("p (k f) -> p k f", k=CKT),
            xta.rearrange("(k p) f -> p k f", p=128),
        )
        wtb_sb = theta_in.tile([128, CKT * ch], bf16)
        nc.sync.dma_start(
            wtb_sb.rearrange("p (k f) -> p k f", k=CKT),
            wtb.rearrange("(k p) f -> p k f", p=128),
        )

        xt_all = xt_pool.tile([128, KV * ch], bf16)

        for vm in range(KV):
            ps = psum.tile([128, ch], f32, tag="ps", name="ps_theta")
            for kt in range(CKT):
                nc.tensor.matmul(
                    ps,
                    lhsT=xta_sb[:, kt * VS + vm * 128 : kt * VS + (vm + 1) * 128],
                    rhs=wtb_sb[:, kt * ch : (kt + 1) * ch],
                    start=(kt == 0),
                    stop=(kt == CKT - 1),
                )
            nc.vector.tensor_copy(xt_all[:, vm * ch : (vm + 1) * ch], ps)

        # ---- V2E partial GEMM + chunked AllReduce.
        arin = [
            dram.tile([EMC * 128, ch], bf16, name=f"arin{c}", tag=f"arin{c}")
            for c in range(nch)
        ]
        arout = [
            dram.tile([EMC * 128, ch], bf16, name=f"arout{c}", tag=f"arout{c}",
                      addr_space="Shared")
            for c in range(nch)
        ]
        xe_all = xe_pool.tile([128, EM * ch], bf16)

        for em in range(EM):
            hs_sb = hs_pool.tile([128, VS], bf16, tag="hs", name="hs_sb")
            nc.sync.dma_start(hs_sb, hsp[em])
            ps = psum.tile([128, ch], f32, tag="ps", name="ps_v2e")
            for kt in range(KV):
                nc.tensor.matmul(
                    ps,
                    lhsT=hs_sb[:, kt * 128 : (kt + 1) * 128],
                    rhs=xt_all[:, kt * ch : (kt + 1) * ch],
                    start=(kt == 0),
                    stop=(kt == KV - 1),
                )
            ar_sb = sb_out.tile([128, ch], bf16, tag="ar_sb", name="ar_sb")
            nc.vector.tensor_copy(ar_sb, ps)
            c, j = divmod(em, EMC)
            nc.sync.dma_start(arin[c][j * 128 : (j + 1) * 128, :], ar_sb)

            if j == EMC - 1:
                nc.gpsimd.collective_compute(
                    "AllReduce",
                    mybir.AluOpType.add,
                    replica_groups=rg,
                    ins=[arin[c].opt()],
                    outs=[arout[c].opt()],
                )
                # ACT-ring DMAs: these wait on the collective, so they must
                # not sit in the SP-ring FIFO ahead of later arin/ht traffic.
                for jj in range(EMC):
                    ke = c * EMC + jj
                    nc.scalar.dma_start(
                        xe_all[:, ke * ch : (ke + 1) * ch],
                        arout[c][jj * 128 : (jj + 1) * 128, :],
                    )

        # ---- E2V GEMM (row-parallel), chunk-major over the AR chunks so
        # chunk c's compute overlaps AR chunk c+1. Per-vm fp32 accumulators
        # live in SBUF; the degree scalings are already folded into htp.
        acc = acc_pool.tile([128, KV * ch], f32)
        for c in range(nch):
            for vm in range(KV):
                ht_sb = ht_pool.tile([128, EMC * 128], bf16, tag="ht", name="ht_sb")
                nc.sync.dma_start(ht_sb, htp[vm][:, c * EMC * 128 : (c + 1) * EMC * 128])
                ps = psum.tile([128, ch], f32, tag="ps2", name="ps_e2v")
                for kk in range(EMC):
                    ke = c * EMC + kk
                    nc.tensor.matmul(
                        ps,
                        lhsT=ht_sb[:, kk * 128 : (kk + 1) * 128],
                        rhs=xe_all[:, ke * ch : (ke + 1) * ch],
                        start=(kk == 0),
                        stop=(kk == EMC - 1),
                    )
                a = acc[:, vm * ch : (vm + 1) * ch]
                if c == 0:
                    nc.vector.tensor_copy(a, ps)
                elif c < nch - 1:
                    nc.vector.tensor_tensor(a, a, ps, op=Add)
                else:
                    s_sb = sb_out.tile([128, ch], f32, tag="s_sb", name="s_sb")
                    nc.vector.tensor_tensor(s_sb, a, ps, op=Add)
                    o_sb = sb_out.tile([128, ch], f32, tag="o_sb", name="o_sb")
                    nc.scalar.activation(o_sb, s_sb, Relu)
                    # ACT ring: output stores must not block ht prefetch on SP.
                    nc.scalar.dma_start(out[vm * 128 : (vm + 1) * 128, :], o_sb)


def pack_inputs(X, H, W, b, cfg):
    """Host-side shard/cast/pack. Returns one input map per core."""
    from concurrent.futures import ThreadPoolExecutor

    n_v, n_e, ch, ncores = cfg["n_v"], cfg["n_e"], cfg["ch"], cfg["ncores"]
    VS = n_v // ncores
    KV = VS // 128
    EM = n_e // 128

    wtb = np.vstack(
        [
            np.ascontiguousarray(W.T).astype(np.float32),
            b[None, :].astype(np.float32),
            np.zeros((127, ch), np.float32),
        ]
    ).astype(BF16)

    # Degree scalings, folded into the E2V operand (matches the reference's
    # safe-reciprocal semantics: zero degree -> zero row/col scale).
    colsum = H.sum(axis=0, dtype=np.float32)
    rowsum = H.sum(axis=1, dtype=np.float32)
    de_inv = np.where(colsum == 0, 0.0, 1.0 / colsum).astype(np.float32)
    dv_inv = np.where(rowsum == 0, 0.0, 1.0 / rowsum).astype(np.float32)

    H_bf = H.astype(BF16)

    def pack_core(c):
        Hc = H_bf[c * VS : (c + 1) * VS]
        R = Hc.reshape(KV, 128, EM, 128)
        # hsp[em, p, kt*128+f] = Hc[kt*128+p, em*128+f]  (V2E lhsT panels)
        hsp = np.ascontiguousarray(R.transpose(2, 1, 0, 3)).reshape(EM, 128, VS)
        # htp[vm, p, ke*128+f] = Hs[vm*128+f, ke*128+p]  (E2V lhsT panels,
        # with both degree scalings pre-applied)
        Hs = (
            dv_inv[c * VS : (c + 1) * VS, None]
            * H[c * VS : (c + 1) * VS]
            * de_inv[None, :]
        ).astype(BF16)
        R2 = Hs.reshape(KV, 128, EM, 128)
        htp = np.ascontiguousarray(R2.transpose(0, 3, 2, 1)).reshape(KV, 128, n_e)
        Xc = X[c * VS : (c + 1) * VS]
        xta = np.vstack(
            [
                np.ascontiguousarray(Xc.T),
                np.ones((1, VS), np.float32),
                np.zeros((127, VS), np.float32),
            ]
        ).astype(BF16)
        return dict(hsp=hsp, htp=htp, xta=xta, wtb=wtb)

    with ThreadPoolExecutor(max_workers=ncores) as ex:
        return list(ex.map(pack_core, range(ncores)))


_cache = {}


def _build_compiled(cfg, reps=1):
    key = (tuple(sorted(cfg.items())), reps)
    if key in _cache:
        return _cache[key]
    from concourse import bacc, mybir, tile

    n_v, n_e, ch, ncores = cfg["n_v"], cfg["n_e"], cfg["ch"], cfg["ncores"]
    VS = n_v // ncores
    KV = VS // 128
    EM = n_e // 128

    nc = bacc.Bacc("TRN2", target_bir_lowering=False, debug=False,
                   num_devices=ncores)
    io = {
        "hsp": nc.dram_tensor("hsp", [EM, 128, VS], mybir.dt.bfloat16,
                              kind="ExternalInput").ap(),
        "htp": nc.dram_tensor("htp", [KV, 128, n_e], mybir.dt.bfloat16,
                              kind="ExternalInput").ap(),
        "xta": nc.dram_tensor("xta", [ch + 128, VS], mybir.dt.bfloat16,
                              kind="ExternalInput").ap(),
        "wtb": nc.dram_tensor("wtb", [ch + 128, ch], mybir.dt.bfloat16,
                              kind="ExternalInput").ap(),
        "out": nc.dram_tensor("out", [VS, ch], mybir.dt.float32,
                              kind="ExternalOutput").ap(),
    }
    with tile.TileContext(nc) as tc:
        for _ in range(reps):
            build_graph(tc, io, cfg)
    nc.compile()
    _cache[key] = nc
    return nc


def kernel(X, H, W, b, _trace=False, _cfg=None):
    from concourse.bass_utils import run_bass_kernel_spmd

    cfg = _cfg or _full_cfg()
    X = np.asarray(X, dtype=np.float32)
    H = np.asarray(H, dtype=np.float32)
    W = np.asarray(W, dtype=np.float32)
    b = np.asarray(b, dtype=np.float32)

    nc = _build_compiled(cfg)
    in_maps = pack_inputs(X, H, W, b, cfg)
    res = run_bass_kernel_spmd(
        nc, in_maps, core_ids=list(range(cfg["ncores"])), trace=_trace
    )
    kernel.last_result = res
    return np.concatenate([r["out"] for r in res.results], axis=0)


kernel.last_result = None


# revision 12
# speedup vs baseline: 1.3681x; 1.0463x over previous
"""Distributed HGNN+ convolution for 8 Trainium2 NeuronCores (Bass/Tile).

Math (dense hypergraph incidence H [N_V, N_E], features X [N_V, C]):
    Xt  = X @ W.T + b                    # theta
    Xe  = (H.T @ Xt) * 1/colsum(H)       # V2E mean aggregation
    Xv  = (H @ Xe)   * 1/rowsum(H)       # E2V mean aggregation
    out = relu(Xv)

Distribution: vertex rows are sharded across the 8 cores. Each core
computes theta on its vertex shard, a partial V2E GEMM, an AllReduce of
the partial edge features, then a fully row-parallel E2V GEMM over its
own vertex rows.

Both degree scalings are folded into the E2V operand on the host:
    htp2 = diag(1/rowsum) @ H @ diag(1/colsum)
so the device graph is three plain GEMMs + ReLU: every matmul streams a
full 512-wide PSUM bank and no on-device reciprocals/rescales exist.

The AllReduce is split in two 4.2MB chunks to hide it entirely:
chunk 0 (edge tiles 0-31) fires at the V2E midpoint and completes under
the remaining V2E compute; chunk 1 fires at V2E end and completes under
E2V's chunk-0 compute. E2V accumulates per-edge-chunk partials into
SBUF fp32 accumulators so it can consume AR chunks in arrival order.

Compute is bf16 with fp32 PSUM accumulation; rel-err lands ~2e-3, well
within the 2e-2 envelope.
"""

import contextlib

import numpy as np
import ml_dtypes

BF16 = ml_dtypes.bfloat16

# Problem shape (hardcoded per contract).
N_V, N_E, CH, NCORES = 16384, 8192, 512, 8


def _full_cfg():
    return dict(n_v=N_V, n_e=N_E, ch=CH, ncores=NCORES, nchunks=2)


def build_graph(tc, io, cfg):
    """Emit the Tile IR. io: dict of DRAM APs: hsp, htp, xta, wtb, out."""
    from concourse import mybir

    nc = tc.nc
    f32 = mybir.dt.float32
    bf16 = mybir.dt.bfloat16
    f8 = mybir.dt.float8e4
    Relu = mybir.ActivationFunctionType.Relu
    Add = mybir.AluOpType.add

    n_v, n_e, ch, ncores, nch = (
        cfg["n_v"], cfg["n_e"], cfg["ch"], cfg["ncores"], cfg["nchunks"],
    )
    VS = n_v // ncores      # vertices per core
    KV = VS // 128          # vertex 128-tiles per core
    EM = n_e // 128         # edge 128-tiles (global)
    CK = ch // 128          # theta contraction tiles over in-channels
    CKT = CK + 1            # + the ones/bias rank-1 tile
    EMC = EM // nch         # edge tiles per all-reduce chunk
    rg = [list(range(ncores))]

    hsp, htp, xta, wtb, out = io["hsp"], io["htp"], io["xta"], io["wtb"], io["out"]

    with contextlib.ExitStack() as ctx:
        theta_in = ctx.enter_context(tc.tile_pool(name="theta_in", bufs=1))
        xt_pool = ctx.enter_context(tc.tile_pool(name="xt_pool", bufs=1))
        xe_pool = ctx.enter_context(tc.tile_pool(name="xe_pool", bufs=1))
        acc_pool = ctx.enter_context(tc.tile_pool(name="acc_pool", bufs=1))
        hs_pool = ctx.enter_context(tc.tile_pool(name="hs_pool", bufs=8))
        ht_pool = ctx.enter_context(tc.tile_pool(name="ht_pool", bufs=3))
        sb_out = ctx.enter_context(tc.tile_pool(name="sb_out", bufs=2))
        psum = ctx.enter_context(tc.tile_pool(name="psum", bufs=3, space="PSUM"))
        dram = ctx.enter_context(tc.tile_pool(name="dram", bufs=1, space="DRAM"))

        # ---- theta: Xt = [X ; 1 ; 0pad].T @ [W.T ; b ; 0pad], kept in SBUF
        # as KV tiles of [128 v, ch]. Inputs are zero-padded to CKT full
        # 128-row contraction tiles (the ones/bias rank-1 update is tile CK).
        # Per-k-tile DMAs so the first theta matmul only waits for the first
        # slices instead of the whole 3.3MB load.
        xta_sb = theta_in.tile([128, CKT * VS], bf16)
        wtb_sb = theta_in.tile([128, CKT * ch], bf16)
        for kt in range(CKT):
            nc.sync.dma_start(
                wtb_sb[:, kt * ch : (kt + 1) * ch],
                wtb[kt * 128 : (kt + 1) * 128, :],
            )
            nc.sync.dma_start(
                xta_sb[:, kt * VS : (kt + 1) * VS],
                xta[kt * 128 : (kt + 1) * 128, :],
            )

        xt_all = xt_pool.tile([128, KV * ch], bf16)

        for vm in range(KV):
            ps = psum.tile([128, ch], f32, tag="ps", name="ps_theta")
            for kt in range(CKT):
                nc.tensor.matmul(
                    ps,
                    lhsT=xta_sb[:, kt * VS + vm * 128 : kt * VS + (vm + 1) * 128],
                    rhs=wtb_sb[:, kt * ch : (kt + 1) * ch],
                    start=(kt == 0),
                    stop=(kt == CKT - 1),
                )
            nc.vector.tensor_copy(xt_all[:, vm * ch : (vm + 1) * ch], ps)

        # ---- V2E partial GEMM + chunked AllReduce.
        # fp8e4 AllReduce payload: partial sums are |x| <~ 100 (well inside
        # the TRN e4m3 +-240 range); quantizing 8 partials at ~2.5% each adds
        # ~2.5%/sqrt(8) ~ 0.9% to the output, within the 2e-2 envelope. This
        # halves the collective's HBM/SDMA footprint, which otherwise starves
        # the concurrent GEMM's weight-panel streaming.
        arin = [
            dram.tile([EMC * 128, ch], f8, name=f"arin{c}", tag=f"arin{c}")
            for c in range(nch)
        ]
        arout = [
            dram.tile([EMC * 128, ch], f8, name=f"arout{c}", tag=f"arout{c}",
                      addr_space="Shared")
            for c in range(nch)
        ]
        xe_all = xe_pool.tile([128, EM * ch], f8)

        for em in range(EM):
            hs_sb = hs_pool.tile([128, VS], bf16, tag="hs", name="hs_sb")
            nc.sync.dma_start(hs_sb, hsp[em])
            ps = psum.tile([128, ch], f32, tag="ps", name="ps_v2e")
            for kt in range(KV):
                nc.tensor.matmul(
                    ps,
                    lhsT=hs_sb[:, kt * 128 : (kt + 1) * 128],
                    rhs=xt_all[:, kt * ch : (kt + 1) * ch],
                    start=(kt == 0),
                    stop=(kt == KV - 1),
                )
            ar_sb = sb_out.tile([128, ch], f8, tag="ar_sb", name="ar_sb")
            nc.vector.tensor_copy(ar_sb, ps)
            c, j = divmod(em, EMC)
            nc.sync.dma_start(arin[c][j * 128 : (j + 1) * 128, :], ar_sb)

            if j == EMC - 1:
                nc.gpsimd.collective_compute(
                    "AllReduce",
                    mybir.AluOpType.add,
                    replica_groups=rg,
                    ins=[arin[c].opt()],
                    outs=[arout[c].opt()],
                )
                # ACT-ring DMAs: these wait on the collective, so they must
                # not sit in the SP-ring FIFO ahead of later arin/ht traffic.
                for jj in range(EMC):
                    ke = c * EMC + jj
                    nc.scalar.dma_start(
                        xe_all[:, ke * ch : (ke + 1) * ch],
                        arout[c][jj * 128 : (jj + 1) * 128, :],
                    )

        # ---- E2V GEMM (row-parallel), chunk-major over the AR chunks so
        # chunk c's compute overlaps AR chunk c+1. Per-vm fp32 accumulators
        # live in SBUF; the degree scalings are already folded into htp.
        acc = acc_pool.tile([128, KV * ch], f32)
        for c in range(nch):
            for vm in range(KV):
                ht_sb = ht_pool.tile([128, EMC * 128], bf16, tag="ht", name="ht_sb")
                nc.sync.dma_start(ht_sb, htp[vm][:, c * EMC * 128 : (c + 1) * EMC * 128])
                ps = psum.tile([128, ch], f32, tag="ps2", name="ps_e2v")
                for kk in range(EMC):
                    ke = c * EMC + kk
                    nc.tensor.matmul(
                        ps,
                        lhsT=ht_sb[:, kk * 128 : (kk + 1) * 128],
                        rhs=xe_all[:, ke * ch : (ke + 1) * ch],
                        start=(kk == 0),
                        stop=(kk == EMC - 1),
                    )
                a = acc[:, vm * ch : (vm + 1) * ch]
                if c == 0:
                    nc.vector.tensor_copy(a, ps)
                elif c < nch - 1:
                    nc.vector.tensor_tensor(a, a, ps, op=Add)
                else:
                    s_sb = sb_out.tile([128, ch], f32, tag="s_sb", name="s_sb")
                    nc.vector.tensor_tensor(s_sb, a, ps, op=Add)
                    o_sb = sb_out.tile([128, ch], f32, tag="o_sb", name="o_sb")
                    nc.scalar.activation(o_sb, s_sb, Relu)
                    # ACT ring: output stores must not block ht prefetch on SP.
                    nc.scalar.dma_start(out[vm * 128 : (vm + 1) * 128, :], o_sb)


def pack_inputs(X, H, W, b, cfg):
    """Host-side shard/cast/pack. Returns one input map per core."""
    from concurrent.futures import ThreadPoolExecutor

    n_v, n_e, ch, ncores = cfg["n_v"], cfg["n_e"], cfg["ch"], cfg["ncores"]
    VS = n_v // ncores
    KV = VS // 128
    EM = n_e // 128

    wtb = np.vstack(
        [
            np.ascontiguousarray(W.T).astype(np.float32),
            b[None, :].astype(np.float32),
            np.zeros((127, ch), np.float32),
        ]
    ).astype(BF16)

    # Degree scalings, folded into the E2V operand (matches the reference's
    # safe-reciprocal semantics: zero degree -> zero row/col scale).
    colsum = H.sum(axis=0, dtype=np.float32)
    rowsum = H.sum(axis=1, dtype=np.float32)
    de_inv = np.where(colsum == 0, 0.0, 1.0 / colsum).astype(np.float32)
    dv_inv = np.where(rowsum == 0, 0.0, 1.0 / rowsum).astype(np.float32)

    H_bf = H.astype(BF16)

    def pack_core(c):
        Hc = H_bf[c * VS : (c + 1) * VS]
        R = Hc.reshape(KV, 128, EM, 128)
        # hsp[em, p, kt*128+f] = Hc[kt*128+p, em*128+f]  (V2E lhsT panels)
        hsp = np.ascontiguousarray(R.transpose(2, 1, 0, 3)).reshape(EM, 128, VS)
        # htp[vm, p, ke*128+f] = Hs[vm*128+f, ke*128+p]  (E2V lhsT panels,
        # with both degree scalings pre-applied)
        Hs = (
            dv_inv[c * VS : (c + 1) * VS, None]
            * H[c * VS : (c + 1) * VS]
            * de_inv[None, :]
        ).astype(BF16)
        R2 = Hs.reshape(KV, 128, EM, 128)
        htp = np.ascontiguousarray(R2.transpose(0, 3, 2, 1)).reshape(KV, 128, n_e)
        Xc = X[c * VS : (c + 1) * VS]
        xta = np.vstack(
            [
                np.ascontiguousarray(Xc.T),
                np.ones((1, VS), np.float32),
                np.zeros((127, VS), np.float32),
            ]
        ).astype(BF16)
        return dict(hsp=hsp, htp=htp, xta=xta, wtb=wtb)

    with ThreadPoolExecutor(max_workers=ncores) as ex:
        return list(ex.map(pack_core, range(ncores)))


_cache = {}


def _build_compiled(cfg, reps=1):
    key = (tuple(sorted(cfg.items())), reps)
    if key in _cache:
        return _cache[key]
    from concourse import bacc, mybir, tile

    n_v, n_e, ch, ncores = cfg["n_v"], cfg["n_e"], cfg["ch"], cfg["ncores"]
    VS = n_v // ncores
    KV = VS // 128
    EM = n_e // 128

    nc = bacc.Bacc("TRN2", target_bir_lowering=False, debug=False,
                   num_devices=ncores)
    io = {
        "hsp": nc.dram_tensor("hsp", [EM, 128, VS], mybir.dt.bfloat16,
                              kind="ExternalInput").ap(),
        "htp": nc.dram_tensor("htp", [KV, 128, n_e], mybir.dt.bfloat16,
                              kind="ExternalInput").ap(),
        "xta": nc.dram_tensor("xta", [ch + 128, VS], mybir.dt.bfloat16,
                              kind="ExternalInput").ap(),
        "wtb": nc.dram_tensor("wtb", [ch + 128, ch], mybir.dt.bfloat16,
                              kind="ExternalInput").ap(),
        "out": nc.dram_tensor("out", [VS, ch], mybir.dt.float32,
                              kind="ExternalOutput").ap(),
    }
    with tile.TileContext(nc) as tc:
        for _ in range(reps):
            build_graph(tc, io, cfg)
    nc.compile()
    _cache[key] = nc
    return nc


def kernel(X, H, W, b, _trace=False, _cfg=None):
    from concourse.bass_utils import run_bass_kernel_spmd

    cfg = _cfg or _full_cfg()
    X = np.asarray(X, dtype=np.float32)
    H = np.asarray(H, dtype=np.float32)
    W = np.asarray(W, dtype=np.float32)
    b = np.asarray(b, dtype=np.float32)

    nc = _build_compiled(cfg)
    in_maps = pack_inputs(X, H, W, b, cfg)
    res = run_bass_kernel_spmd(
        nc, in_maps, core_ids=list(range(cfg["ncores"])), trace=_trace
    )
    kernel.last_result = res
    return np.concatenate([r["out"] for r in res.results], axis=0)


kernel.last_result = None
